# revision 1
# baseline (speedup 1.0000x reference)
"""DevignModel (GGNN message passing) Trainium2 kernel, 8 NeuronCores.

Strategy (graph/edge-cut parallelism per the sharding hint):
  - Nodes sharded contiguously across 8 cores (12544 padded rows each);
    h kept feature-major (h^T, bf16) resident in SBUF.
  - Per GGNN step: m = h @ W_t computed per 128-node tile on the tensor
    engine, written bf16 to a DRAM bounce buffer, AllGather'd so every
    core holds the full 100352-row message table.
  - Edges partitioned by dst core and dst tile-pair, bucketed by src
    segment (25088 rows so relative indices fit dma_gather's int16), and
    fetched with dma_gather on 4 SWDGE queues. Scatter-add is one-hot
    selection matmuls accumulating in PSUM (S[e,d] = (dst_rel[e]==d)
    built on DVE).
  - GRU gates run fp32 on PE/ACT/DVE; h^T updated in place per tile.
  - Epilogue: per-graph sums via one-hot matmul into PSUM; host adds the
    8 partial [128,385] blocks, divides by counts, and runs the tiny MLP.
"""

import numpy as np
import ml_dtypes

import concourse.bass as bass
import concourse.bacc as bacc
import concourse.mybir as mybir
import concourse.tile as tile
from concourse import bass_utils, library_config

F32 = mybir.dt.float32
BF16 = mybir.dt.bfloat16
I16 = mybir.dt.int16

CORES = 8
P = 128
HID = 256
IN_DIM = 128
GATE3 = 768


def _default_cfg():
    return dict(
        NREAL=100000,
        E=3200000,
        STEPS=6,
        NGRAPH=128,
        NLOC=12800,  # padded nodes per core (multiple of 128)
        NSEG=5,      # src segments = sub-shard AllGather pieces
    )


def _derived(cfg):
    c = dict(cfg)
    c["NLOCREAL"] = c["NREAL"] // CORES
    c["TILES"] = c["NLOC"] // P
    c["NPAD"] = c["NLOC"] * CORES
    c["SEGROWS"] = c["NPAD"] // c["NSEG"]
    c["SUBSH"] = c["NLOC"] // c["NSEG"]
    assert c["SEGROWS"] <= 32768
    assert c["NLOC"] % P == 0 and c["NLOC"] % c["NSEG"] == 0
    assert c["SUBSH"] % P == 0 and c["TILES"] % c["NSEG"] == 0
    return c


def preprocess(x, edge_index, batch, cfg):
    """Build per-core and shared device arrays. Returns (arrays, meta)."""
    c = _derived(cfg)
    NLR, NLOC, T, NSEG, SEGROWS = (
        c["NLOCREAL"], c["NLOC"], c["TILES"], c["NSEG"], c["SEGROWS"])

    x = np.asarray(x, dtype=np.float32)
    src = np.asarray(edge_index[0], dtype=np.int64)
    dst = np.asarray(edge_index[1], dtype=np.int64)
    batch = np.asarray(batch, dtype=np.int64)

    # real node id -> padded id
    s_pad = (src // NLR) * NLOC + src % NLR
    d_pad = (dst // NLR) * NLOC + dst % NLR

    ecore = d_pad // NLOC
    tloc = (d_pad % NLOC) // P
    drel = (d_pad % P).astype(np.float32)
    # sub-shard table layout: table_k = concat over ranks of each rank's
    # k-th SUBSH-row slice; seg k of src row = (loc // SUBSH)
    SUBSH = c["SUBSH"]
    s_rank = s_pad // NLOC
    s_loc = s_pad % NLOC
    seg = (s_loc // SUBSH).astype(np.int64)
    srel = (s_rank * SUBSH + s_loc % SUBSH).astype(np.int64)

    nbuck = CORES * T * NSEG
    bucket = (ecore * T + tloc) * NSEG + seg
    order = np.argsort(bucket, kind="stable")
    counts = np.bincount(bucket, minlength=nbuck)
    starts = np.zeros(nbuck + 1, dtype=np.int64)
    np.cumsum(counts, out=starts[1:])
    pos = np.arange(len(src)) - starts[bucket[order]]

    # per-(tile, seg) chunk count = max over cores, >= 1
    cc = counts.reshape(CORES, T, NSEG)
    ch_ts = np.maximum(1, (cc.max(axis=0) + P - 1) // P)  # [T, NSEG]
    cap_ts = ch_ts * P
    ch_flat = ch_ts.reshape(-1)
    cap_off = np.zeros(T * NSEG + 1, dtype=np.int64)
    np.cumsum(cap_ts.reshape(-1), out=cap_off[1:])
    CAPSUM = int(cap_off[-1])

    idx_cap = np.zeros((CORES, CAPSUM), dtype=np.int16)
    drel_cap = np.full((CORES, CAPSUM), 200.0, dtype=np.float32)
    bs = bucket[order]
    b_core = bs // (T * NSEG)
    b_ts = bs % (T * NSEG)
    flatpos = cap_off[b_ts] + pos
    idx_cap[b_core, flatpos] = srel[order].astype(np.int16)
    drel_cap[b_core, flatpos] = drel[order]

    # pack idx: bucket b occupies cols [cap_off[b]//16, cap_off[b+1]//16);
    # within the bucket, flat index i -> [16*g + i%16, i//16] for g in 0..8
    idx_packed = np.zeros((CORES, P, CAPSUM // 16), dtype=np.int16)
    v16 = idx_cap.reshape(CORES, CAPSUM // 16, 16)  # cap_off multiples of 128
    tmp = v16.transpose(0, 2, 1)  # [C, 16, CAPSUM//16]
    idx_packed[:] = np.tile(tmp, (1, 8, 1))

    # pack dstrel: bucket b cols [choff[b], choff[b+1]); [p, j] = drel[j*128+p]
    ch_off = np.zeros(T * NSEG + 1, dtype=np.int64)
    np.cumsum(ch_flat, out=ch_off[1:])
    CHSUM = int(ch_off[-1])
    dr_packed = drel_cap.reshape(CORES, CHSUM, P).transpose(0, 2, 1)
    dr_packed = dr_packed.astype(ml_dtypes.bfloat16)

    # x per core, padded
    x_loc = np.zeros((CORES, NLOC, IN_DIM), dtype=np.float32)
    x_loc[:, :NLR] = x.reshape(CORES, NLR, IN_DIM)
    xr = x_loc.reshape(CORES, T, P, IN_DIM)
    xaug = np.ones((CORES, T, P, IN_DIM + 1), dtype=np.float32)
    xaug[..., :IN_DIM] = xr

    # h0^T resident layout: [128, T*256]; cols t*256 + k*128 + n
    h0t = np.zeros((CORES, P, T, 2, P), dtype=np.float32)
    h0t[:, :, :, 0, :] = xr.transpose(0, 3, 1, 2)
    h0t = h0t.reshape(CORES, P, T * HID)

    # batch rel, pad rows excluded with 200
    b_loc = np.full((CORES, NLOC), 200.0, dtype=np.float32)
    b_loc[:, :NLR] = batch.reshape(CORES, NLR).astype(np.float32)
    batchrel = b_loc.reshape(CORES, T, P).transpose(0, 2, 1)  # [C, 128, T]

    arrays = dict(
        idx=idx_packed, dstrel=dr_packed, xaug=xaug, h0t=h0t,
        batchrel=batchrel,
    )
    meta = dict(cfg=c, ch_ts=ch_ts, cap_off=cap_off, ch_off=ch_off,
                CAPSUM=CAPSUM, CHSUM=CHSUM)
    return arrays, meta


def shared_arrays(ggnn_weight, w_ih, w_hh, b_ih, b_hh, steps):
    bf = ml_dtypes.bfloat16
    gg = np.asarray(ggnn_weight, np.float32).reshape(steps, 2, P, HID)
    w_all = gg.transpose(2, 0, 1, 3).reshape(P, steps * 2 * HID).copy()
    wihT = np.asarray(w_ih, np.float32).T.reshape(2, P, GATE3).copy()
    whhT = np.asarray(w_hh, np.float32).T.reshape(2, P, GATE3).copy()
    b_ih = np.asarray(b_ih, np.float32)
    b_hh = np.asarray(b_hh, np.float32)
    return dict(
        w_all=w_all, wihT=wihT, whhT=whhT,
        b_rz=(b_ih + b_hh)[None, :512].copy(),
        b_in=b_ih[None, 512:].copy(),
        b_hn=b_hh[None, 512:].copy(),
        ones1=np.ones((1, P), np.float32),
        iota_bf=np.tile(np.arange(P, dtype=np.float32), (P, 1)).astype(bf),
        iota_f32=np.tile(np.arange(P, dtype=np.float32), (P, 1)),
        ident=np.eye(P, dtype=np.float32),
    )


def _gather_calls(ch):
    """Split a ch-chunk region into dma_gather calls of <=8 chunks."""
    calls, c0 = [], 0
    while c0 < ch:
        n = min(8, ch - c0)
        calls.append((c0, n))
        c0 += n
    return calls


def build(meta):
    c = meta["cfg"]
    T, NSEG, SEGROWS, NLOC, NPAD, STEPS = (
        c["TILES"], c["NSEG"], c["SEGROWS"], c["NLOC"], c["NPAD"], c["STEPS"])
    SUBSH = c["SUBSH"]
    TPS = T // NSEG  # tiles per sub-shard
    ch_ts = meta["ch_ts"]  # [T, NSEG]
    cap_off, ch_off = meta["cap_off"], meta["ch_off"]
    CAPSUM, CHSUM = meta["CAPSUM"], meta["CHSUM"]
    CSMAX = int(ch_ts.max())

    nc = bacc.Bacc("TRN2", target_bir_lowering=False, debug=False,
                   num_devices=CORES, num_swdge_queues=4)

    idx_d = nc.dram_tensor("idx", [P, CAPSUM // 16], I16, kind="ExternalInput")
    dr_d = nc.dram_tensor("dstrel", [P, CHSUM], BF16, kind="ExternalInput")
    xaug_d = nc.dram_tensor("xaug", [T, P, IN_DIM + 1], F32, kind="ExternalInput")
    h0t_d = nc.dram_tensor("h0t", [P, T * HID], F32, kind="ExternalInput")
    batch_d = nc.dram_tensor("batchrel", [P, T], F32, kind="ExternalInput")
    wall_d = nc.dram_tensor("w_all", [P, STEPS * 2 * HID], F32,
                            kind="ExternalInput")
    wihT_d = nc.dram_tensor("wihT", [2, P, GATE3], F32, kind="ExternalInput")
    whhT_d = nc.dram_tensor("whhT", [2, P, GATE3], F32, kind="ExternalInput")
    brz_d = nc.dram_tensor("b_rz", [1, 512], F32, kind="ExternalInput")
    bin_d = nc.dram_tensor("b_in", [1, HID], F32, kind="ExternalInput")
    bhn_d = nc.dram_tensor("b_hn", [1, HID], F32, kind="ExternalInput")
    ones_d = nc.dram_tensor("ones1", [1, P], F32, kind="ExternalInput")
    iotab_d = nc.dram_tensor("iota_bf", [P, P], BF16, kind="ExternalInput")
    iotaf_d = nc.dram_tensor("iota_f32", [P, P], F32, kind="ExternalInput")
    ident_d = nc.dram_tensor("ident", [P, P], F32, kind="ExternalInput")
    pool_d = nc.dram_tensor("pool", [P, IN_DIM + 1 + HID], F32,
                            kind="ExternalOutput")

    with tile.TileContext(nc) as tc:
        with (
            tc.tile_pool(name="const", bufs=1) as cpool,
            tc.tile_pool(name="stream", bufs=3) as stpool,
            tc.tile_pool(name="gq", bufs=12) as gpool,
            tc.tile_pool(name="Sp", bufs=2) as spool,
            tc.tile_pool(name="work", bufs=2) as wpool,
            tc.tile_pool(name="dram", bufs=1, space="DRAM") as dpool,
        ):
            nc.gpsimd.load_library(library_config.mlp)

            def load_const(name, dram, shape, dtype):
                t_ = cpool.tile(shape, dtype, name=name)
                nc.sync.dma_start(out=t_[:], in_=dram)
                return t_

            wih_sb = [load_const(f"wih{k}", wihT_d[k, :, :], [P, GATE3], F32)
                      for k in range(2)]
            whh_sb = [load_const(f"whh{k}", whhT_d[k, :, :], [P, GATE3], F32)
                      for k in range(2)]
            brz_sb = load_const("brz", brz_d[:, :], [1, 512], F32)
            bin_sb = load_const("bin", bin_d[:, :], [1, HID], F32)
            bhn_sb = load_const("bhn", bhn_d[:, :], [1, HID], F32)
            ones_sb = load_const("ones", ones_d[:, :], [1, P], F32)
            iotab_sb = load_const("iotab", iotab_d[:, :], [P, P], BF16)
            iotaf_sb = load_const("iotaf", iotaf_d[:, :], [P, P], F32)
            ident_sb = load_const("ident", ident_d[:, :], [P, P], F32)
            batch_sb = load_const("batch", batch_d[:, :], [P, T], F32)

            hT = []
            for i in range(T):
                t_ = cpool.tile([P, HID], F32, name=f"hT{i}")
                nc.sync.dma_start(out=t_[:], in_=h0t_d[:, i * HID:(i + 1) * HID])
                hT.append(t_)

            swdge_ctr = [0]  # aligns queue_num with Tile's DMASW lane RR

            cc_ins = [[dpool.tile([SUBSH, HID], BF16, name=f"cc_in{t}_{k}")
                       for k in range(NSEG)] for t in range(STEPS)]
            cc_outs = [[dpool.tile([SEGROWS, HID], BF16,
                                   name=f"cc_out{t}_{k}", addr_space="Shared")
                        for k in range(NSEG)] for t in range(STEPS)]

            hfin = dpool.tile([NLOC, HID], F32, name="hfin")

            def gru_tile(t, i, agg_ps, scrps, rzps, npsp):
                """Everything after aggregation for dst tile i at step t."""
                agg_sb = wpool.tile([P, HID], F32, tag="aggsb",
                                    name=f"aggsb{t}_{i}")
                nc.scalar.copy(out=agg_sb[:], in_=agg_ps[:])
                aggT_ps = scrps.tile([P, HID], F32, tag="scr",
                                     name=f"aggT{t}_{i}")
                nc.tensor.transpose(out=aggT_ps[:, 0:P],
                                    in_=agg_sb[:, 0:P], identity=ident_sb[:])
                nc.tensor.transpose(out=aggT_ps[:, P:HID],
                                    in_=agg_sb[:, P:HID], identity=ident_sb[:])
                aggT_sb = wpool.tile([P, HID], F32, tag="aggT",
                                     name=f"aggTs{t}_{i}")
                nc.scalar.copy(out=aggT_sb[:], in_=aggT_ps[:])

                hnat_ps = scrps.tile([P, HID], F32, tag="scr",
                                     name=f"hnat{t}_{i}")
                nc.tensor.transpose(out=hnat_ps[:, 0:P],
                                    in_=hT[i][:, 0:P], identity=ident_sb[:])
                nc.tensor.transpose(out=hnat_ps[:, P:HID],
                                    in_=hT[i][:, P:HID], identity=ident_sb[:])
                hnat_sb = wpool.tile([P, HID], F32, tag="hnat",
                                     name=f"hnats{t}_{i}")
                nc.scalar.copy(out=hnat_sb[:], in_=hnat_ps[:])

                rz_ps = rzps.tile([P, 512], F32, tag="rz", name=f"rz{t}_{i}")
                nc.tensor.matmul(out=rz_ps[:], lhsT=aggT_sb[:, 0:P],
                                 rhs=wih_sb[0][:, 0:512], start=True, stop=False)
                nc.tensor.matmul(out=rz_ps[:], lhsT=aggT_sb[:, P:HID],
                                 rhs=wih_sb[1][:, 0:512], start=False, stop=False)
                nc.tensor.matmul(out=rz_ps[:], lhsT=hT[i][:, 0:P],
                                 rhs=whh_sb[0][:, 0:512], start=False, stop=False)
                nc.tensor.matmul(out=rz_ps[:], lhsT=hT[i][:, P:HID],
                                 rhs=whh_sb[1][:, 0:512], start=False, stop=False)
                nc.tensor.matmul(out=rz_ps[:], lhsT=ones_sb[:],
                                 rhs=brz_sb[:], start=False, stop=True)
                rz_sb = wpool.tile([P, 512], F32, tag="rzsb", name=f"rzsb{t}_{i}")
                nc.scalar.activation(
                    out=rz_sb[:], in_=rz_ps[:],
                    func=mybir.ActivationFunctionType.Sigmoid)

                n_ps = npsp.tile([P, 512], F32, tag="n", name=f"n{t}_{i}")
                nc.tensor.matmul(out=n_ps[:, 0:HID], lhsT=aggT_sb[:, 0:P],
                                 rhs=wih_sb[0][:, 512:GATE3],
                                 start=True, stop=False)
                nc.tensor.matmul(out=n_ps[:, 0:HID], lhsT=aggT_sb[:, P:HID],
                                 rhs=wih_sb[1][:, 512:GATE3],
                                 start=False, stop=False)
                nc.tensor.matmul(out=n_ps[:, 0:HID], lhsT=ones_sb[:],
                                 rhs=bin_sb[:], start=False, stop=True)
                nc.tensor.matmul(out=n_ps[:, HID:512], lhsT=hT[i][:, 0:P],
                                 rhs=whh_sb[0][:, 512:GATE3],
                                 start=True, stop=False)
                nc.tensor.matmul(out=n_ps[:, HID:512], lhsT=hT[i][:, P:HID],
                                 rhs=whh_sb[1][:, 512:GATE3],
                                 start=False, stop=False)
                nc.tensor.matmul(out=n_ps[:, HID:512], lhsT=ones_sb[:],
                                 rhs=bhn_sb[:], start=False, stop=True)

                t1_sb = wpool.tile([P, HID], F32, tag="tmp", name=f"t1{t}_{i}", bufs=4)
                nc.vector.tensor_tensor(out=t1_sb[:], in0=rz_sb[:, 0:HID],
                                        in1=n_ps[:, HID:512],
                                        op=mybir.AluOpType.mult)
                t2_sb = wpool.tile([P, HID], F32, tag="tmp", name=f"t2{t}_{i}", bufs=4)
                nc.vector.tensor_tensor(out=t2_sb[:], in0=t1_sb[:],
                                        in1=n_ps[:, 0:HID],
                                        op=mybir.AluOpType.add)
                nn_sb = wpool.tile([P, HID], F32, tag="nn", name=f"nn{t}_{i}")
                nc.scalar.activation(out=nn_sb[:], in_=t2_sb[:],
                                     func=mybir.ActivationFunctionType.Tanh)
                d_sb = wpool.tile([P, HID], F32, tag="tmp", name=f"d{t}_{i}", bufs=4)
                nc.vector.tensor_sub(out=d_sb[:], in0=hnat_sb[:], in1=nn_sb[:])
                zd_sb = wpool.tile([P, HID], F32, tag="tmp", name=f"zd{t}_{i}", bufs=4)
                nc.vector.tensor_mul(out=zd_sb[:], in0=rz_sb[:, HID:512],
                                     in1=d_sb[:])
                hnew_sb = wpool.tile([P, HID], F32, tag="hnew",
                                     name=f"hnew{t}_{i}")
                nc.vector.tensor_add(out=hnew_sb[:], in0=zd_sb[:], in1=nn_sb[:])

                if t == STEPS - 1:
                    nc.sync.dma_start(out=hfin[i * P:(i + 1) * P, :],
                                      in_=hnew_sb[:])
                else:
                    hTn_ps = scrps.tile([P, HID], F32, tag="scr",
                                        name=f"hTn{t}_{i}")
                    nc.tensor.transpose(out=hTn_ps[:, 0:P],
                                        in_=hnew_sb[:, 0:P],
                                        identity=ident_sb[:])
                    nc.tensor.transpose(out=hTn_ps[:, P:HID],
                                        in_=hnew_sb[:, P:HID],
                                        identity=ident_sb[:])
                    nc.scalar.copy(out=hT[i][:], in_=hTn_ps[:])

            with (
                tc.tile_pool(name="scrps", bufs=3, space="PSUM") as scrps,
                tc.tile_pool(name="aggps", bufs=3, space="PSUM") as aggps,
                tc.tile_pool(name="rzps", bufs=1, space="PSUM") as rzps,
                tc.tile_pool(name="nps", bufs=1, space="PSUM") as npsp,
            ):
                for t in range(STEPS):
                    woff = t * 2 * HID
                    wt_sb = stpool.tile([P, 2 * HID], F32, tag="wt",
                                        name=f"wt{t}")
                    nc.sync.dma_start(out=wt_sb[:],
                                      in_=wall_d[:, woff:woff + 2 * HID])
                    for i in range(T):
                        m_ps = scrps.tile([P, HID], F32, tag="scr",
                                          name=f"mps{t}_{i}")
                        nc.tensor.matmul(
                            out=m_ps[:], lhsT=hT[i][:, 0:P],
                            rhs=wt_sb[:, 0:HID],
                            start=True, stop=False)
                        nc.tensor.matmul(
                            out=m_ps[:], lhsT=hT[i][:, P:HID],
                            rhs=wt_sb[:, HID:2 * HID],
                            start=False, stop=True)
                        m_sb = wpool.tile([P, HID], BF16, tag="m",
                                          name=f"m{t}_{i}")
                        nc.scalar.copy(out=m_sb[:], in_=m_ps[:])
                        k = i // TPS
                        ioff = (i % TPS) * P
                        nc.sync.dma_start(
                            out=cc_ins[t][k][ioff:ioff + P, :], in_=m_sb[:])
                        if i % TPS == TPS - 1:
                            nc.gpsimd.collective_compute(
                                "AllGather", mybir.AluOpType.bypass,
                                replica_groups=[list(range(CORES))],
                                ins=[cc_ins[t][k].opt()],
                                outs=[cc_outs[t][k].opt()])

                    for i in range(T):
                        b0 = i * NSEG
                        icols0 = int(cap_off[b0] // 16)
                        icols1 = int(cap_off[b0 + NSEG] // 16)
                        idx_sb = stpool.tile([P, icols1 - icols0], I16,
                                             tag="idx", name=f"idx{t}_{i}")
                        nc.sync.dma_start(out=idx_sb[:],
                                          in_=idx_d[:, icols0:icols1])
                        dr0 = int(ch_off[b0])
                        dr1 = int(ch_off[b0 + NSEG])
                        dr_sb = stpool.tile([P, dr1 - dr0], BF16,
                                            tag="dr", name=f"dr{t}_{i}")
                        nc.sync.dma_start(out=dr_sb[:], in_=dr_d[:, dr0:dr1])

                        chunk_sl, s_list = [], []
                        for s in range(NSEG):
                            b = b0 + s
                            cs = int(ch_ts[i, s])
                            bic0 = int(cap_off[b] // 16) - icols0
                            slices = []
                            for (c0, ncall) in _gather_calls(cs):
                                g_sb = gpool.tile(
                                    [P, 8 * HID], BF16, tag="g",
                                    name=f"g{t}_{i}_{s}_{c0}")
                                nc.gpsimd.dma_gather(
                                    g_sb[:, 0:ncall * HID]
                                    .rearrange("p (j d) -> p j d", d=HID),
                                    cc_outs[t][s][:, :],
                                    idx_sb[:, bic0 + c0 * 8:
                                           bic0 + (c0 + ncall) * 8],
                                    ncall * P, ncall * P, HID,
                                    queue_num=swdge_ctr[0] % 4)
                                swdge_ctr[0] += 1
                                for jj in range(ncall):
                                    slices.append(
                                        g_sb[:, jj * HID:(jj + 1) * HID])
                            S_sb = spool.tile([P, CSMAX * P], BF16,
                                              tag="S", name=f"S{t}_{i}_{s}")
                            bdr0 = int(ch_off[b]) - dr0
                            nc.vector.tensor_tensor(
                                out=S_sb[:, :cs * P]
                                .rearrange("p (j d) -> p j d", d=P),
                                in0=dr_sb[:, bdr0:bdr0 + cs]
                                .to_broadcast([P, cs, P]),
                                in1=iotab_sb[:]
                                .rearrange("p (a b) -> p a b", a=1)
                                .to_broadcast([P, cs, P]),
                                op=mybir.AluOpType.is_equal)
                            chunk_sl.append(slices)
                            s_list.append(S_sb)

                        agg_ps = aggps.tile([P, HID], F32, tag="agg",
                                            name=f"agg{t}_{i}")
                        nchunks = int(ch_ts[i].sum())
                        k = 0
                        for s in range(NSEG):
                            for j in range(int(ch_ts[i, s])):
                                nc.tensor.matmul(
                                    out=agg_ps[:],
                                    lhsT=s_list[s][:, j * P:(j + 1) * P],
                                    rhs=chunk_sl[s][j],
                                    start=(k == 0), stop=(k == nchunks - 1))
                                k += 1
                        gru_tile(t, i, agg_ps, scrps, rzps, npsp)

            # ---- phase C: per-graph pooling partials ----
            with (
                tc.tile_pool(name="poolps", bufs=1, space="PSUM") as pps,
            ):
                px_ps = pps.tile([P, IN_DIM + 1], F32, name="px")
                ph_ps = pps.tile([P, HID], F32, name="ph")
                for i in range(T):
                    hc_sb = wpool.tile([P, HID], F32, tag="hc", name=f"hcs{i}")
                    nc.sync.dma_start(out=hc_sb[:],
                                      in_=hfin[i * P:(i + 1) * P, :])
                    sg_sb = wpool.tile([P, P], F32, tag="sg", name=f"sg{i}")
                    nc.vector.tensor_scalar(
                        out=sg_sb[:], in0=iotaf_sb[:],
                        scalar1=batch_sb[:, i:i + 1], scalar2=None,
                        op0=mybir.AluOpType.is_equal)
                    xo_sb = stpool.tile([P, IN_DIM + 1], F32, tag="xo",
                                        name=f"xo{i}")
                    nc.sync.dma_start(out=xo_sb[:], in_=xaug_d[i, :, :])
                    nc.tensor.matmul(out=px_ps[:], lhsT=sg_sb[:], rhs=xo_sb[:],
                                     start=(i == 0), stop=(i == T - 1))
                    nc.tensor.matmul(out=ph_ps[:], lhsT=sg_sb[:], rhs=hc_sb[:],
                                     start=(i == 0), stop=(i == T - 1))
                px_sb = wpool.tile([P, IN_DIM + 1], F32, name="pxs")
                nc.vector.tensor_copy(out=px_sb[:], in_=px_ps[:])
                ph_sb = wpool.tile([P, HID], F32, name="phs")
                nc.vector.tensor_copy(out=ph_sb[:], in_=ph_ps[:])
                nc.sync.dma_start(out=pool_d[:, 0:IN_DIM + 1], in_=px_sb[:])
                nc.sync.dma_start(out=pool_d[:, IN_DIM + 1:], in_=ph_sb[:])

    nc.compile()
    return nc


_CACHE = {}


def run_device(inputs, cfg, trace=False, tmpdir=None):
    """Preprocess, compile (cached), run on 8 cores; returns per-core pool
    partials [CORES, 128, 385] plus the BassKernelResults."""
    c = _derived(cfg)
    arrays, meta = preprocess(
        inputs["x"], inputs["edge_index"], inputs["batch"], cfg)
    shared = shared_arrays(
        inputs["ggnn_weight"], inputs["w_ih"], inputs["w_hh"],
        inputs["b_ih"], inputs["b_hh"], c["STEPS"])

    key = (tuple(meta["ch_ts"].reshape(-1).tolist()), c["STEPS"], c["NLOC"])
    if key not in _CACHE:
        _CACHE[key] = build(meta)
    nc = _CACHE[key]

    in_maps = []
    for core in range(CORES):
        m = {k: np.ascontiguousarray(v[core]) for k, v in arrays.items()}
        m.update(shared)
        in_maps.append(m)
    kw = {}
    if trace:
        kw = dict(trace=True, tmpdir=tmpdir)
    res = bass_utils.run_bass_kernel_spmd(
        nc, in_maps, core_ids=list(range(CORES)), **kw)
    pool = np.stack([res.results[cr]["pool"] for cr in range(CORES)])
    return pool, res


def host_epilogue(pool, inputs):
    """Sum per-core partials, mean-pool, and run the classifier MLP."""
    tot = pool.sum(axis=0, dtype=np.float64).astype(np.float32)
    xsum = tot[:, :IN_DIM]
    cnt = tot[:, IN_DIM]
    hsum = tot[:, IN_DIM + 1:]
    feat = np.concatenate([xsum, hsum], axis=1)
    pooled = feat / np.maximum(cnt, 1.0)[:, None]
    w1 = np.asarray(inputs["mlp_w1"], np.float32)
    b1 = np.asarray(inputs["mlp_b1"], np.float32)
    w2 = np.asarray(inputs["mlp_w2"], np.float32)
    b2 = np.asarray(inputs["mlp_b2"], np.float32)
    hdn = np.maximum(pooled @ w1.T + b1, 0.0)
    return (hdn @ w2.T + b2).astype(np.float32)


def kernel(**inputs):
    cfg = _default_cfg()
    pool, _ = run_device(inputs, cfg)
    return host_epilogue(pool, inputs)



# revision 13
# speedup vs baseline: 1.0997x; 1.0997x over previous
"""DevignModel (GGNN message passing) Trainium2 kernel, 8 NeuronCores.

Strategy (graph/edge-cut parallelism per the sharding hint):
  - Nodes sharded contiguously across 8 cores (12800 padded rows each);
    h kept feature-major (h^T) resident in SBUF, fp32, two dst tiles
    paired per SBUF tile: cols k*256 + u*128 + n (k feat half, u tile in
    pair, n node).
  - Per GGNN step: per dst tile, edge messages fetched with dma_gather
    (4 SWDGE queues) from the AllGather'd bf16 message table; scatter-add
    is one-hot matmuls flipped to produce agg^T directly in PSUM, paired.
  - GRU runs in the transposed (feature-major) domain on 256-node-wide
    fp32r matmuls (full PE rate at fp32 precision); biases folded into
    ACT sigmoid/tanh; blend on DVE in fp32. No PE transposes in-loop.
  - m(t+1) = h(t+1) @ W_{t+1} computed right after each tile's GRU
    update (fp32r), rounded to bf16 for the message table; per-segment
    AllGathers fire mid-step and hide under the gather/compute phase.
  - Final step fuses the per-graph mean-pool partials (one-hot matmul,
    bf16); host sums the 8 partial [128,385] blocks and runs the MLP.
"""

import numpy as np
import ml_dtypes

import concourse.bass as bass
import concourse.bacc as bacc
import concourse.mybir as mybir
import concourse.tile as tile
from concourse import bass_utils, library_config

F32 = mybir.dt.float32
F32R = mybir.dt.float32r
BF16 = mybir.dt.bfloat16
I16 = mybir.dt.int16

CORES = 8
P = 128
HID = 256
IN_DIM = 128
GATE3 = 768


def _default_cfg():
    return dict(
        NREAL=100000,
        E=3200000,
        STEPS=6,
        NGRAPH=128,
        NLOC=12800,  # padded nodes per core (multiple of 256)
        NSEG=5,      # src segments = sub-shard AllGather pieces
    )


def _derived(cfg):
    c = dict(cfg)
    c["NLOCREAL"] = c["NREAL"] // CORES
    c["TILES"] = c["NLOC"] // P
    c["NPAD"] = c["NLOC"] * CORES
    c["SEGROWS"] = c["NPAD"] // c["NSEG"]
    c["SUBSH"] = c["NLOC"] // c["NSEG"]
    assert c["SEGROWS"] <= 32768
    assert c["NLOC"] % 256 == 0 and c["NLOC"] % c["NSEG"] == 0
    assert c["SUBSH"] % P == 0 and c["TILES"] % c["NSEG"] == 0
    assert c["TILES"] % 2 == 0
    return c


def preprocess(x, edge_index, batch, cfg):
    """Build per-core and shared device arrays. Returns (arrays, meta)."""
    c = _derived(cfg)
    NLR, NLOC, T, NSEG, SEGROWS = (
        c["NLOCREAL"], c["NLOC"], c["TILES"], c["NSEG"], c["SEGROWS"])

    bf = ml_dtypes.bfloat16
    x = np.asarray(x, dtype=np.float32)
    src = np.asarray(edge_index[0], dtype=np.int64)
    dst = np.asarray(edge_index[1], dtype=np.int64)
    batch = np.asarray(batch, dtype=np.int64)

    # real node id -> padded id
    s_pad = (src // NLR) * NLOC + src % NLR
    d_pad = (dst // NLR) * NLOC + dst % NLR

    ecore = d_pad // NLOC
    tloc = (d_pad % NLOC) // P
    drel = (d_pad % P).astype(np.float32)
    # sub-shard table layout: table_k = concat over ranks of each rank's
    # k-th SUBSH-row slice; seg k of src row = (loc // SUBSH)
    SUBSH = c["SUBSH"]
    s_rank = s_pad // NLOC
    s_loc = s_pad % NLOC
    seg = (s_loc // SUBSH).astype(np.int64)
    srel = (s_rank * SUBSH + s_loc % SUBSH).astype(np.int64)

    nbuck = CORES * T * NSEG
    bucket = (ecore * T + tloc) * NSEG + seg
    order = np.argsort(bucket, kind="stable")
    counts = np.bincount(bucket, minlength=nbuck)
    starts = np.zeros(nbuck + 1, dtype=np.int64)
    np.cumsum(counts, out=starts[1:])
    pos = np.arange(len(src)) - starts[bucket[order]]

    # per-(tile, seg) chunk count = max over cores, >= 1
    cc = counts.reshape(CORES, T, NSEG)
    ch_ts = np.maximum(1, (cc.max(axis=0) + P - 1) // P)  # [T, NSEG]
    cap_ts = ch_ts * P
    ch_flat = ch_ts.reshape(-1)
    cap_off = np.zeros(T * NSEG + 1, dtype=np.int64)
    np.cumsum(cap_ts.reshape(-1), out=cap_off[1:])
    CAPSUM = int(cap_off[-1])

    idx_cap = np.zeros((CORES, CAPSUM), dtype=np.int16)
    drel_cap = np.full((CORES, CAPSUM), 200.0, dtype=np.float32)
    bs = bucket[order]
    b_core = bs // (T * NSEG)
    b_ts = bs % (T * NSEG)
    flatpos = cap_off[b_ts] + pos
    idx_cap[b_core, flatpos] = srel[order].astype(np.int16)
    drel_cap[b_core, flatpos] = drel[order]

    # pack idx: bucket b occupies cols [cap_off[b]//16, cap_off[b+1]//16);
    # within the bucket, flat index i -> [16*g + i%16, i//16] for g in 0..8
    idx_packed = np.zeros((CORES, P, CAPSUM // 16), dtype=np.int16)
    v16 = idx_cap.reshape(CORES, CAPSUM // 16, 16)  # cap_off multiples of 128
    tmp = v16.transpose(0, 2, 1)  # [C, 16, CAPSUM//16]
    idx_packed[:] = np.tile(tmp, (1, 8, 1))

    # pack dstrel: bucket b cols [choff[b], choff[b+1]); [p, j] = drel[j*128+p]
    ch_off = np.zeros(T * NSEG + 1, dtype=np.int64)
    np.cumsum(ch_flat, out=ch_off[1:])
    CHSUM = int(ch_off[-1])
    dr_packed = drel_cap.reshape(CORES, CHSUM, P).transpose(0, 2, 1)
    dr_packed = dr_packed.astype(bf)

    # x per core, padded; bf16 with trailing ones column for counts
    x_loc = np.zeros((CORES, NLOC, IN_DIM), dtype=np.float32)
    x_loc[:, :NLR] = x.reshape(CORES, NLR, IN_DIM)
    xr = x_loc.reshape(CORES, T, P, IN_DIM)
    xaug = np.ones((CORES, T, P, IN_DIM + 1), dtype=np.float32)
    xaug[..., :IN_DIM] = xr
    xaug = xaug.astype(bf)

    # h0^T paired fp32 layout: [128, (T//2)*512];
    # col i2*512 + k*256 + u*128 + n = h0[(2*i2+u)*128+n, k*128+f]
    h0t = np.zeros((CORES, P, T // 2, 2, 2, P), dtype=np.float32)
    xr2 = xr.reshape(CORES, T // 2, 2, P, IN_DIM)  # [C, i2, u, n, f]
    h0t[:, :, :, 0, :, :] = xr2.transpose(0, 4, 1, 2, 3)
    h0t = h0t.reshape(CORES, P, (T // 2) * 2 * HID)

    # batch rel, pad rows excluded with 200
    b_loc = np.full((CORES, NLOC), 200.0, dtype=np.float32)
    b_loc[:, :NLR] = batch.reshape(CORES, NLR).astype(np.float32)
    batchrel = b_loc.reshape(CORES, T, P).transpose(0, 2, 1)

    arrays = dict(
        idx=idx_packed, dstrel=dr_packed, xaug=xaug, h0t=h0t,
        batchrel=batchrel,
    )
    meta = dict(cfg=c, ch_ts=ch_ts, cap_off=cap_off, ch_off=ch_off,
                CAPSUM=CAPSUM, CHSUM=CHSUM)
    return arrays, meta


def shared_arrays(ggnn_weight, w_ih, w_hh, b_ih, b_hh, steps):
    bf = ml_dtypes.bfloat16
    gg = np.asarray(ggnn_weight, np.float32).reshape(steps, 2, P, HID)
    w_all = gg.transpose(2, 0, 1, 3).reshape(P, steps * 2 * HID)
    wihT = np.asarray(w_ih, np.float32).T.reshape(2, P, GATE3)
    whhT = np.asarray(w_hh, np.float32).T.reshape(2, P, GATE3)
    b_ih = np.asarray(b_ih, np.float32)
    b_hh = np.asarray(b_hh, np.float32)
    brz = (b_ih + b_hh)[:512].reshape(4, P).T.copy()   # [128, 4]
    bin_c = b_ih[512:].reshape(2, P).T.copy()          # [128, 2]
    bhn_c = b_hh[512:].reshape(2, P).T.copy()          # [128, 2]
    return dict(
        w_all=np.ascontiguousarray(w_all),
        wihT=np.ascontiguousarray(wihT),
        whhT=np.ascontiguousarray(whhT),
        b_rz=brz, b_in=bin_c, b_hn=bhn_c,
        iota_bf=np.tile(np.arange(P, dtype=np.float32), (P, 1)).astype(bf),
        ident_f32=np.eye(P, dtype=np.float32),
    )


def _gather_calls(ch):
    """Split a ch-chunk region into dma_gather calls of <=8 chunks."""
    calls, c0 = [], 0
    while c0 < ch:
        n = min(8, ch - c0)
        calls.append((c0, n))
        c0 += n
    return calls


def build(meta):
    c = meta["cfg"]
    T, NSEG, SEGROWS, NLOC, NPAD, STEPS = (
        c["TILES"], c["NSEG"], c["SEGROWS"], c["NLOC"], c["NPAD"], c["STEPS"])
    SUBSH = c["SUBSH"]
    TPS = T // NSEG  # tiles per sub-shard
    PAIRS = T // 2
    ch_ts = meta["ch_ts"]  # [T, NSEG]
    cap_off, ch_off = meta["cap_off"], meta["ch_off"]
    CAPSUM, CHSUM = meta["CAPSUM"], meta["CHSUM"]
    CSMAX = int(ch_ts.max())
    NGB = 9  # gather ring buffers

    nc = bacc.Bacc("TRN2", target_bir_lowering=False, debug=False,
                   num_devices=CORES, num_swdge_queues=4)

    idx_d = nc.dram_tensor("idx", [P, CAPSUM // 16], I16, kind="ExternalInput")
    dr_d = nc.dram_tensor("dstrel", [P, CHSUM], BF16, kind="ExternalInput")
    xaug_d = nc.dram_tensor("xaug", [T, P, IN_DIM + 1], BF16,
                            kind="ExternalInput")
    h0t_d = nc.dram_tensor("h0t", [P, (T // 2) * 2 * HID], F32R,
                           kind="ExternalInput")
    batch_d = nc.dram_tensor("batchrel", [P, T], F32, kind="ExternalInput")
    wall_d = nc.dram_tensor("w_all", [P, STEPS * 2 * HID], F32R,
                            kind="ExternalInput")
    wihT_d = nc.dram_tensor("wihT", [2, P, GATE3], F32R, kind="ExternalInput")
    whhT_d = nc.dram_tensor("whhT", [2, P, GATE3], F32R, kind="ExternalInput")
    brz_d = nc.dram_tensor("b_rz", [P, 4], F32, kind="ExternalInput")
    bin_d = nc.dram_tensor("b_in", [P, 2], F32, kind="ExternalInput")
    bhn_d = nc.dram_tensor("b_hn", [P, 2], F32, kind="ExternalInput")
    iotab_d = nc.dram_tensor("iota_bf", [P, P], BF16, kind="ExternalInput")
    identf_d = nc.dram_tensor("ident_f32", [P, P], F32R, kind="ExternalInput")
    pool_d = nc.dram_tensor("pool", [P, IN_DIM + 1 + HID], F32,
                            kind="ExternalOutput")

    with tile.TileContext(nc) as tc:
        with (
            tc.tile_pool(name="const", bufs=1) as cpool,
            tc.tile_pool(name="stream", bufs=3) as stpool,
            tc.tile_pool(name="Sp", bufs=6) as spool,
            tc.tile_pool(name="work", bufs=2) as wpool,
            tc.tile_pool(name="dram", bufs=1, space="DRAM") as dpool,
        ):
            nc.gpsimd.load_library(library_config.mlp)

            def load_const(name, dram, shape, dtype):
                t_ = cpool.tile(shape, dtype, name=name)
                nc.sync.dma_start(out=t_[:], in_=dram)
                return t_

            wih_sb = [load_const(f"wih{k}", wihT_d[k, :, :], [P, GATE3], F32R)
                      for k in range(2)]
            whh_sb = [load_const(f"whh{k}", whhT_d[k, :, :], [P, GATE3], F32R)
                      for k in range(2)]
            brz_sb = load_const("brz", brz_d[:, :], [P, 4], F32)
            bin_sb = load_const("bin", bin_d[:, :], [P, 2], F32)
            bhn_sb = load_const("bhn", bhn_d[:, :], [P, 2], F32)
            iotab_sb = load_const("iotab", iotab_d[:, :], [P, P], BF16)
            identf_sb = load_const("identf", identf_d[:, :], [P, P], F32R)
            batch_sb = load_const("batch", batch_d[:, :], [P, T], F32)

            hT2 = []
            for i2 in range(PAIRS):
                t_ = cpool.tile([P, 2 * HID], F32R, name=f"hT2_{i2}")
                nc.sync.dma_start(
                    out=t_[:], in_=h0t_d[:, i2 * 2 * HID:(i2 + 1) * 2 * HID])
                hT2.append(t_)

            # static gather ring (reused buffers; consumed slices are
            # always freshly written by their gather)
            gring = []
            for j in range(NGB):
                g_ = cpool.tile([P, 8 * HID], BF16, name=f"gring{j}")
                gring.append(g_)
            gctr = [0]
            swdge_ctr = [0]

            cc_ins = [[dpool.tile([SUBSH, HID], BF16, name=f"cc_in{t}_{k}")
                       for k in range(NSEG)] for t in range(STEPS)]
            cc_outs = [[dpool.tile([SEGROWS, HID], BF16,
                                   name=f"cc_out{t}_{k}", addr_space="Shared")
                        for k in range(NSEG)] for t in range(STEPS)]

            with (
                tc.tile_pool(name="scrps", bufs=3, space="PSUM") as scrps,
                tc.tile_pool(name="rzps", bufs=1, space="PSUM") as rzps,
                tc.tile_pool(name="nps", bufs=1, space="PSUM") as npsp,
                tc.tile_pool(name="poolps", bufs=1, space="PSUM") as pps,
            ):
                def hcol(u, k):
                    """hT2 col offset for (tile-in-pair u, feat half k)."""
                    return k * HID + u * P

                wt_cur = [None]

                def load_wt(t):
                    wt_ = stpool.tile([P, 2 * HID], F32R, tag="wt",
                                      name=f"wt{t}", bufs=2)
                    nc.sync.dma_start(
                        out=wt_[:], in_=wall_d[:, t * 2 * HID:(t + 1) * 2 * HID])
                    wt_cur[0] = wt_

                def emit_m(t, i):
                    """m(t) for tile i from current hT2; fire AG at seg end."""
                    i2, u = i // 2, i % 2
                    m_ps = scrps.tile([P, HID], F32, tag="scr",
                                      name=f"mps{t}_{i}")
                    wt_ = wt_cur[0]
                    for k in range(2):
                        nc.tensor.matmul(
                            out=m_ps[:],
                            lhsT=hT2[i2][:, hcol(u, k):hcol(u, k) + P],
                            rhs=wt_[:, k * HID:(k + 1) * HID],
                            start=(k == 0), stop=(k == 1))
                    m_sb = wpool.tile([P, HID], BF16, tag="m", name=f"m{t}_{i}")
                    nc.scalar.copy(out=m_sb[:], in_=m_ps[:])
                    kseg = i // TPS
                    ioff = (i % TPS) * P
                    nc.sync.dma_start(
                        out=cc_ins[t][kseg][ioff:ioff + P, :], in_=m_sb[:])
                    if i % TPS == TPS - 1:
                        nc.gpsimd.collective_compute(
                            "AllGather", mybir.AluOpType.bypass,
                            replica_groups=[list(range(CORES))],
                            ins=[cc_ins[t][kseg].opt()],
                            outs=[cc_outs[t][kseg].opt()])

                # pooling accumulator in SBUF (used at t = STEPS-1)
                pxh_acc = cpool.tile([P, IN_DIM + 1 + HID], F32, name="pxhacc")
                nc.vector.memset(pxh_acc[:], 0.0)

                # prologue: m(0) for all tiles
                load_wt(0)
                for i in range(T):
                    emit_m(0, i)

                def gather_tile(t, i):
                    """Gathers + S build for dst tile i."""
                    b0 = i * NSEG
                    icols0 = int(cap_off[b0] // 16)
                    icols1 = int(cap_off[b0 + NSEG] // 16)
                    idx_sb = stpool.tile([P, icols1 - icols0], I16,
                                         tag="idx", name=f"idx{t}_{i}")
                    nc.sync.dma_start(out=idx_sb[:],
                                      in_=idx_d[:, icols0:icols1])
                    dr0 = int(ch_off[b0])
                    dr1 = int(ch_off[b0 + NSEG])
                    dr_sb = stpool.tile([P, dr1 - dr0], BF16,
                                        tag="dr", name=f"dr{t}_{i}")
                    nc.sync.dma_start(out=dr_sb[:], in_=dr_d[:, dr0:dr1])

                    chunk_sl, s_list = [], []
                    for s in range(NSEG):
                        b = b0 + s
                        cs = int(ch_ts[i, s])
                        bic0 = int(cap_off[b] // 16) - icols0
                        slices = []
                        for (c0, ncall) in _gather_calls(cs):
                            g_sb = gring[gctr[0] % NGB]
                            gctr[0] += 1
                            nc.gpsimd.dma_gather(
                                g_sb[:, 0:ncall * HID]
                                .rearrange("p (j d) -> p j d", d=HID),
                                cc_outs[t][s][:, :],
                                idx_sb[:, bic0 + c0 * 8:
                                       bic0 + (c0 + ncall) * 8],
                                ncall * P, ncall * P, HID,
                                queue_num=swdge_ctr[0] % 4)
                            swdge_ctr[0] += 1
                            for jj in range(ncall):
                                slices.append(
                                    g_sb[:, jj * HID:(jj + 1) * HID])
                        S_sb = spool.tile([P, CSMAX * P], BF16,
                                          tag="S", name=f"S{t}_{i}_{s}")
                        bdr0 = int(ch_off[b]) - dr0
                        nc.vector.tensor_tensor(
                            out=S_sb[:, :cs * P]
                            .rearrange("p (j d) -> p j d", d=P),
                            in0=dr_sb[:, bdr0:bdr0 + cs]
                            .to_broadcast([P, cs, P]),
                            in1=iotab_sb[:]
                            .rearrange("p (a b) -> p a b", a=1)
                            .to_broadcast([P, cs, P]),
                            op=mybir.AluOpType.is_equal)
                        chunk_sl.append(slices)
                        s_list.append(S_sb)
                    return chunk_sl, s_list

                for t in range(STEPS):
                    if t < STEPS - 1:
                        load_wt(t + 1)
                    for i2 in range(PAIRS):
                        # paired agg^T in PSUM: cols k*256 + u*128 + n
                        aggT_ps = scrps.tile([P, 2 * HID], F32, tag="scr",
                                             name=f"agg{t}_{i2}")
                        for u in range(2):
                            i = 2 * i2 + u
                            chunk_sl, s_list = gather_tile(t, i)
                            nchunks = int(ch_ts[i].sum())
                            for k in range(2):
                                kk = 0
                                oc = hcol(u, k)
                                for s in range(NSEG):
                                    for j in range(int(ch_ts[i, s])):
                                        nc.tensor.matmul(
                                            out=aggT_ps[:, oc:oc + P],
                                            lhsT=chunk_sl[s][j][:, k * P:
                                                                (k + 1) * P],
                                            rhs=s_list[s][:, j * P:(j + 1) * P],
                                            start=(kk == 0),
                                            stop=(kk == nchunks - 1))
                                        kk += 1

                        aggT_sb = wpool.tile([P, 2 * HID], F32R, tag="aggT",
                                             name=f"aggTs{t}_{i2}")
                        nc.scalar.copy(out=aggT_sb[:], in_=aggT_ps[:])

                        # gates (transposed domain, fp32r, 256-node wide)
                        rz_ps = rzps.tile([P, 1024], F32, tag="rz",
                                          name=f"rz{t}_{i2}")
                        for cch in range(4):
                            oc = cch * 2 * P
                            wc = cch * P
                            nc.tensor.matmul(
                                out=rz_ps[:, oc:oc + 2 * P],
                                lhsT=wih_sb[0][:, wc:wc + P],
                                rhs=aggT_sb[:, 0:HID],
                                start=True, stop=False)
                            nc.tensor.matmul(
                                out=rz_ps[:, oc:oc + 2 * P],
                                lhsT=wih_sb[1][:, wc:wc + P],
                                rhs=aggT_sb[:, HID:2 * HID],
                                start=False, stop=False)
                            nc.tensor.matmul(
                                out=rz_ps[:, oc:oc + 2 * P],
                                lhsT=whh_sb[0][:, wc:wc + P],
                                rhs=hT2[i2][:, 0:HID],
                                start=False, stop=False)
                            nc.tensor.matmul(
                                out=rz_ps[:, oc:oc + 2 * P],
                                lhsT=whh_sb[1][:, wc:wc + P],
                                rhs=hT2[i2][:, HID:2 * HID],
                                start=False, stop=True)
                        rz_sb = wpool.tile([P, 1024], F32, tag="rzsb",
                                           name=f"rzsb{t}_{i2}")
                        for cch in range(4):
                            oc = cch * 2 * P
                            nc.scalar.activation(
                                out=rz_sb[:, oc:oc + 2 * P],
                                in_=rz_ps[:, oc:oc + 2 * P],
                                func=mybir.ActivationFunctionType.Sigmoid,
                                bias=brz_sb[:, cch:cch + 1])

                        n_ps = npsp.tile([P, 1024], F32, tag="n",
                                         name=f"n{t}_{i2}")
                        for c2 in range(2):
                            oc = c2 * 2 * P
                            wc = (4 + c2) * P
                            nc.tensor.matmul(
                                out=n_ps[:, oc:oc + 2 * P],
                                lhsT=wih_sb[0][:, wc:wc + P],
                                rhs=aggT_sb[:, 0:HID],
                                start=True, stop=False)
                            nc.tensor.matmul(
                                out=n_ps[:, oc:oc + 2 * P],
                                lhsT=wih_sb[1][:, wc:wc + P],
                                rhs=aggT_sb[:, HID:2 * HID],
                                start=False, stop=True)
                            nc.tensor.matmul(
                                out=n_ps[:, 512 + oc:512 + oc + 2 * P],
                                lhsT=whh_sb[0][:, wc:wc + P],
                                rhs=hT2[i2][:, 0:HID],
                                start=True, stop=False)
                            nc.tensor.matmul(
                                out=n_ps[:, 512 + oc:512 + oc + 2 * P],
                                lhsT=whh_sb[1][:, wc:wc + P],
                                rhs=hT2[i2][:, HID:2 * HID],
                                start=False, stop=True)

                        # n = tanh(i_n + b_in + r*(h_n + b_hn))
                        hnb_sb = wpool.tile([P, 512], F32, tag="hnb",
                                            name=f"hnb{t}_{i2}")
                        for c2 in range(2):
                            oc = c2 * 2 * P
                            nc.vector.tensor_scalar(
                                out=hnb_sb[:, oc:oc + 2 * P],
                                in0=n_ps[:, 512 + oc:512 + oc + 2 * P],
                                scalar1=bhn_sb[:, c2:c2 + 1], scalar2=None,
                                op0=mybir.AluOpType.add)
                        t1_sb = wpool.tile([P, 512], F32, tag="t1",
                                           name=f"t1{t}_{i2}")
                        nc.vector.tensor_tensor(
                            out=t1_sb[:], in0=rz_sb[:, 0:512], in1=hnb_sb[:],
                            op=mybir.AluOpType.mult)
                        nc.vector.tensor_tensor(
                            out=n_ps[:, 0:512], in0=n_ps[:, 0:512],
                            in1=t1_sb[:], op=mybir.AluOpType.add)
                        n_sb = wpool.tile([P, 512], F32, tag="nn",
                                          name=f"nn{t}_{i2}")
                        for c2 in range(2):
                            oc = c2 * 2 * P
                            nc.scalar.activation(
                                out=n_sb[:, oc:oc + 2 * P],
                                in_=n_ps[:, oc:oc + 2 * P],
                                func=mybir.ActivationFunctionType.Tanh,
                                bias=bin_sb[:, c2:c2 + 1])

                        # h' = n + z*(h - n), transposed paired layout
                        d_sb = wpool.tile([P, 512], F32, tag="d",
                                          name=f"d{t}_{i2}")
                        nc.vector.tensor_sub(out=d_sb[:], in0=hT2[i2][:],
                                             in1=n_sb[:])
                        zd_sb = wpool.tile([P, 512], F32, tag="zd",
                                           name=f"zd{t}_{i2}")
                        nc.vector.tensor_mul(out=zd_sb[:],
                                             in0=rz_sb[:, 512:1024],
                                             in1=d_sb[:])
                        nc.vector.tensor_add(out=hT2[i2][:], in0=zd_sb[:],
                                             in1=n_sb[:])

                        if t < STEPS - 1:
                            emit_m(t + 1, 2 * i2)
                            emit_m(t + 1, 2 * i2 + 1)
                        else:
                            for u in range(2):
                                i = 2 * i2 + u
                                hnat_ps = scrps.tile([P, HID], F32R, tag="scr",
                                                     name=f"hnat{i}")
                                for k in range(2):
                                    nc.tensor.transpose(
                                        out=hnat_ps[:, k * P:(k + 1) * P],
                                        in_=hT2[i2][:, hcol(u, k):
                                                    hcol(u, k) + P],
                                        identity=identf_sb[:])
                                hnat_sb = wpool.tile([P, HID], BF16,
                                                     tag="hnat",
                                                     name=f"hnats{i}")
                                nc.scalar.copy(out=hnat_sb[:], in_=hnat_ps[:])
                                sg_sb = wpool.tile([P, P], BF16, tag="sg",
                                                   name=f"sg{i}")
                                nc.vector.tensor_scalar(
                                    out=sg_sb[:], in0=iotab_sb[:],
                                    scalar1=batch_sb[:, i:i + 1], scalar2=None,
                                    op0=mybir.AluOpType.is_equal)
                                xo_sb = stpool.tile([P, IN_DIM + 1], BF16,
                                                    tag="xo", name=f"xo{i}")
                                nc.sync.dma_start(out=xo_sb[:],
                                                  in_=xaug_d[i, :, :])
                                pxh_ps = pps.tile([P, IN_DIM + 1 + HID], F32,
                                                  tag="pp", name=f"pxh{i}")
                                nc.tensor.matmul(
                                    out=pxh_ps[:, 0:IN_DIM + 1], lhsT=sg_sb[:],
                                    rhs=xo_sb[:], start=True, stop=True)
                                nc.tensor.matmul(
                                    out=pxh_ps[:, IN_DIM + 1:], lhsT=sg_sb[:],
                                    rhs=hnat_sb[:], start=True, stop=True)
                                nc.vector.tensor_add(out=pxh_acc[:],
                                                     in0=pxh_acc[:],
                                                     in1=pxh_ps[:])

                nc.sync.dma_start(out=pool_d[:, :], in_=pxh_acc[:])

    nc.compile()
    return nc


_CACHE = {}


def run_device(inputs, cfg, trace=False, tmpdir=None):
    """Preprocess, compile (cached), run on 8 cores; returns per-core pool
    partials [CORES, 128, 385] plus the BassKernelResults."""
    c = _derived(cfg)
    arrays, meta = preprocess(
        inputs["x"], inputs["edge_index"], inputs["batch"], cfg)
    shared = shared_arrays(
        inputs["ggnn_weight"], inputs["w_ih"], inputs["w_hh"],
        inputs["b_ih"], inputs["b_hh"], c["STEPS"])

    key = (tuple(meta["ch_ts"].reshape(-1).tolist()), c["STEPS"], c["NLOC"])
    if key not in _CACHE:
        _CACHE[key] = build(meta)
    nc = _CACHE[key]

    in_maps = []
    for core in range(CORES):
        m = {k: np.ascontiguousarray(v[core]) for k, v in arrays.items()}
        m.update(shared)
        in_maps.append(m)
    kw = {}
    if trace:
        kw = dict(trace=True, tmpdir=tmpdir)
    res = bass_utils.run_bass_kernel_spmd(
        nc, in_maps, core_ids=list(range(CORES)), **kw)
    pool = np.stack([res.results[cr]["pool"] for cr in range(CORES)])
    return pool, res


def host_epilogue(pool, inputs):
    """Sum per-core partials, mean-pool, and run the classifier MLP."""
    tot = pool.sum(axis=0, dtype=np.float64).astype(np.float32)
    xsum = tot[:, :IN_DIM]
    cnt = tot[:, IN_DIM]
    hsum = tot[:, IN_DIM + 1:]
    feat = np.concatenate([xsum, hsum], axis=1)
    pooled = feat / np.maximum(cnt, 1.0)[:, None]
    w1 = np.asarray(inputs["mlp_w1"], np.float32)
    b1 = np.asarray(inputs["mlp_b1"], np.float32)
    w2 = np.asarray(inputs["mlp_w2"], np.float32)
    b2 = np.asarray(inputs["mlp_b2"], np.float32)
    hdn = np.maximum(pooled @ w1.T + b1, 0.0)
    return (hdn @ w2.T + b2).astype(np.float32)


def kernel(**inputs):
    cfg = _default_cfg()
    pool, _ = run_device(inputs, cfg)
    return host_epilogue(pool, inputs)
